# revision 23
# baseline (speedup 1.0000x reference)
"""Trainium2 Bass kernel for nn_FFNet_17600775979626.

Spiking FFN layer: cur = einsum('tbi,oi->tbo', x, W) + b, followed by a
leaky-integrate-and-fire scan over T with subtractive reset (snntorch Leaky,
beta=0.95, threshold=1.0). Returns spk_rec [T, B, NO] (0.0/1.0 floats).

Distribution: output-neuron sharding. Each of the 8 cores computes all
(T, B) for a 256-wide slice of the 2048 output neurons. x is replicated
(transposed on host so the contraction dim lands on SBUF partitions); W^T
and b are sliced per core. The per-timestep GEMM accumulates 16 K-chunk
matmuls (+ a K=1 bias seed matmul) into a PSUM tile laid out [b=128, o=256],
which is exactly the layout the membrane scan needs, so the scan reads PSUM
directly with full-width DVE ops and spikes DMA out contiguously.

Recurrence (TH=1): m_t = w_{t-1} + cur_t;  spk_t = (m_t > 1);
                   w_t = beta*m_t - spk_t.

Walrus codegen on this target accepts at most ONE sync-wait command per
engine instruction, while Tile's wait assigner freely emits several. Two
post-scheduling passes fix that: _slim_waits drops waits already implied
transitively (per-queue FIFO dispatch + semaphore vector clocks), and
_split_waits moves any excess waits onto injected same-queue NoOps.
"""

import os

import numpy as np

T, B, NI, NO = 128, 128, 2048, 2048
NCORES = 8
O_S = NO // NCORES  # 256 output neurons per core
KC = NI // 128  # 16 contraction chunks
BETA = 0.95

# GEMM modes:
#  fp16x2: x and W split into fp16 (hi, lo) pairs on host; three 1-cycle/row
#          passes xh*Wh + xh*Wl + xl*Wh. fp16 products are exact in fp32, so
#          accuracy matches native fp32 while streaming 3x faster.
#  fp32:   native fp32 matmuls (4 cycles/row).
#  f32r:   relaxed fp32, 1 cycle/row but ~tf32 multiply precision (~3e-2 rel
#          error on this problem's spikes) - too lossy, kept for experiments.
MODE = os.environ.get("KERNEL_MODE", "fp16x2")

_cache = {}


def _build_nc(mode):
    from contextlib import ExitStack

    import concourse.bass as bass
    import concourse.mybir as mybir
    import concourse.tile as tile

    f32 = mybir.dt.float32
    split16 = mode == "fp16x2"
    if split16:
        dt_mm = mybir.dt.float16
    elif mode == "f32r":
        dt_mm = mybir.dt.float32r
    else:
        dt_mm = f32

    nc = bass.Bass()
    # xT planes: fp16x2 ships (hi, lo); other modes use plane 0 only
    n_planes = 2 if split16 else 1
    xT = nc.declare_dram_parameter("xT", [n_planes, NI, T * B], dt_mm, isOutput=False)
    WTs = nc.declare_dram_parameter("WTs", [n_planes, NI, O_S], dt_mm, isOutput=False)
    # cols 0..127: ones (lhsT of K=1 bias seed matmuls); then bias plane(s)
    ob = nc.declare_dram_parameter(
        "ob", [1, 128 + n_planes * O_S], dt_mm, isOutput=False
    )
    spk = nc.declare_dram_parameter("spk", [T, B, O_S], f32, isOutput=True)

    TQ = 4  # timesteps per DMA batch (>=512B contiguous runs, fewer DMAs)
    assert T % TQ == 0

    with tile.TileContext(nc) as tc, ExitStack() as ctx:
        singles = ctx.enter_context(tc.tile_pool(name="singles", bufs=1))
        xpool = ctx.enter_context(tc.tile_pool(name="xp", bufs=2))
        spool = ctx.enter_context(tc.tile_pool(name="sp", bufs=3))
        psum = ctx.enter_context(tc.tile_pool(name="ps", bufs=6, space="PSUM"))

        # W^T resident in SBUF: [i%128, plane, i//128, o]
        wt_sb = singles.tile([128, n_planes, KC, O_S], dt_mm)
        nc.sync.dma_start(
            out=wt_sb[:], in_=WTs[:].rearrange("h (k p) o -> p h k o", p=128)
        )

        ob_sb = singles.tile([1, 128 + n_planes * O_S], dt_mm)
        nc.sync.dma_start(out=ob_sb[:], in_=ob[:])

        m_sb = singles.tile([128, O_S], f32)  # membrane potential
        w_sb = singles.tile([128, O_S], f32)  # carry: beta*m - spk
        nc.vector.memset(w_sb[:], 0.0)

        xTr = xT[:].rearrange("h (k p) tb -> p h k tb", p=128)
        spk_r = spk[:].rearrange("(tq tt) b o -> tq b tt o", tt=TQ)

        for tq in range(T // TQ):
            xt = xpool.tile([128, n_planes, KC, TQ * B], dt_mm)
            nc.sync.dma_start(
                out=xt[:], in_=xTr[:, :, :, tq * TQ * B : (tq + 1) * TQ * B]
            )
            st = spool.tile([128, TQ, O_S], f32)

            for tt in range(TQ):
                ps = psum.tile([128, O_S], f32)
                first = True
                for k in range(KC):
                    if split16:
                        # xh*Wh, xh*Wl (shared stationary xh), then xl*Wh
                        passes = ((0, 0), (0, 1), (1, 0))
                    else:
                        passes = ((0, 0),)
                    for hx, hw in passes:
                        nc.tensor.matmul(
                            ps[:],
                            lhsT=xt[:, hx, k, tt * B : (tt + 1) * B],
                            rhs=wt_sb[:, hw, k, :],
                            start=first,
                            stop=False,
                        )
                        first = False
                for h in range(n_planes):
                    nc.tensor.matmul(
                        ps[:],
                        lhsT=ob_sb[:, :128],
                        rhs=ob_sb[:, 128 + h * O_S : 128 + (h + 1) * O_S],
                        start=False,
                        stop=(h == n_planes - 1),
                    )

                nc.vector.tensor_tensor(m_sb[:], w_sb[:], ps[:], mybir.AluOpType.add)
                nc.vector.tensor_scalar(
                    st[:, tt, :], m_sb[:], 1.0, None, mybir.AluOpType.is_gt
                )
                nc.vector.scalar_tensor_tensor(
                    w_sb[:],
                    m_sb[:],
                    BETA,
                    st[:, tt, :],
                    mybir.AluOpType.mult,
                    mybir.AluOpType.subtract,
                )
            # one store per TQ steps: dst [b part, tt, o] view of spk[tq]
            nc.sync.dma_start(out=spk_r[tq], in_=st[:])

    _slim_waits(nc)
    _split_waits(nc)
    return nc


def _slim_waits(nc):
    """Drop sync waits already implied by earlier ones (transitive closure).

    Each engine queue dispatches in FIFO order, so a wait satisfied on an
    earlier instruction of the same queue covers later instructions. A wait
    on sem s >= v also imports everything the incrementing instruction's
    queue had itself waited for when it raised s to v (semaphore vector
    clocks with snapshots at each increment).
    """
    FRAMEWORK_OPS = ("InstEventSemaphore", "InstDrain")
    engine_clock = {}  # engine -> {sem_id: value known reached}
    totals = {}  # sem_id -> running total of increments
    snapshots = {}  # sem_id -> [(value, clock dict)] in increasing value order
    poisoned = set()  # sems touched by non-monotonic updates (barriers)

    def join(dst, src):
        for s, v in src.items():
            if s in poisoned:
                continue
            if dst.get(s, -1) < v:
                dst[s] = v

    for blk in nc.m.functions[0].blocks:
        for inst in blk.instructions:
            si = getattr(inst, "sync_info", None)
            if si is None:
                continue
            is_framework = type(inst).__name__ in FRAMEWORK_OPS
            clock = engine_clock.setdefault(inst.engine, {})
            if si.on_wait:
                kept = []
                for w in si.on_wait:
                    if (
                        w.sync_type != "semaphore"
                        or w.wait_mode != "sem-ge-imm"
                        or w.id in poisoned
                    ):
                        kept.append(w)
                        continue
                    covered = clock.get(w.id, -1) >= w.wait_value
                    for val, snap in snapshots.get(w.id, ()):
                        if val <= w.wait_value:
                            join(clock, snap)
                        else:
                            break
                    if clock.get(w.id, -1) < w.wait_value:
                        clock[w.id] = w.wait_value
                    if is_framework or not covered:
                        kept.append(w)
                si.on_wait = kept
            if si.on_update:
                for u in si.on_update:
                    if u.sync_type != "semaphore":
                        continue
                    if u.update_mode not in ("sem-inc", "sem-add-imm"):
                        # barrier-style sem: stop reasoning about it entirely
                        poisoned.add(u.id)
                        totals.pop(u.id, None)
                        snapshots.pop(u.id, None)
                        for c in engine_clock.values():
                            c.pop(u.id, None)
                        continue
                    if u.id in poisoned:
                        continue
                    tot = totals.get(u.id, 0) + (u.update_value or 1)
                    totals[u.id] = tot
                    snap = dict(clock)
                    snap[u.id] = tot
                    snapshots.setdefault(u.id, []).append((tot, snap))


def _split_waits(nc, limit=1):
    """Move excess sync waits onto injected same-queue NoOps.

    Walrus codegen accepts at most `limit` sync-wait commands per engine
    instruction on this target. Engine queues dispatch in order, so a
    preceding NoOp carrying the wait is equivalent.
    """
    import concourse.mybir as mybir

    n_nops = 0
    for blk in nc.m.functions[0].blocks:
        out = []
        changed = False
        for inst in blk.instructions:
            si = getattr(inst, "sync_info", None)
            if type(inst).__name__ == "InstEventSemaphore":
                out.append(inst)
                continue
            if si is not None and si.on_wait and len(si.on_wait) > limit:
                waits = list(si.on_wait)
                for w in waits[:-limit]:
                    nop = mybir.InstNoOp(name=f"wnop-{n_nops}", ins=[], outs=[])
                    n_nops += 1
                    nop.engine = inst.engine
                    nop.sync_info = mybir.SyncInfo(on_wait=[w], on_update=[])
                    nop.bass_nofuse = True
                    out.append(nop)
                    changed = True
                si.on_wait = waits[-limit:]
            out.append(inst)
        if changed:
            try:
                blk.instructions = out
            except Exception:
                blk.instructions.clear()
                blk.instructions.extend(out)


def _split16(a):
    hi = a.astype(np.float16)
    lo = (a - hi.astype(np.float32)).astype(np.float16)
    return hi, lo


def _prepare_in_maps(x, W, b):
    x = np.ascontiguousarray(x, dtype=np.float32)
    W = np.ascontiguousarray(W, dtype=np.float32)
    b = np.ascontiguousarray(b, dtype=np.float32)
    # row tb = t*B + b so a 128-column block of xT = one full timestep
    x2 = x.reshape(T * B, NI)
    if MODE == "fp16x2":
        xh, xl = _split16(x2)
        xT = np.stack([np.ascontiguousarray(xh.T), np.ascontiguousarray(xl.T)])
        Wh, Wl = _split16(W)
        WTs_full = np.stack([np.ascontiguousarray(Wh.T), np.ascontiguousarray(Wl.T)])
        bh, bl = _split16(b)
        b_planes = [bh, bl]
        npdt = np.float16
    else:
        xT = np.ascontiguousarray(x2.T)[None]
        WTs_full = np.ascontiguousarray(W.T)[None]
        b_planes = [b]
        npdt = np.float32
    n_planes = len(b_planes)
    in_maps = []
    for c in range(NCORES):
        ob = np.empty((1, 128 + n_planes * O_S), npdt)
        ob[0, :128] = 1.0
        for h in range(n_planes):
            ob[0, 128 + h * O_S : 128 + (h + 1) * O_S] = b_planes[h][
                c * O_S : (c + 1) * O_S
            ]
        in_maps.append(
            {
                "xT": xT,
                "WTs": np.ascontiguousarray(WTs_full[:, :, c * O_S : (c + 1) * O_S]),
                "ob": ob,
            }
        )
    return in_maps


def run(x, W, b, trace=False):
    """Run the kernel; returns (out [T,B,NO] fp32, BassKernelResults)."""
    from concourse.bass_utils import run_bass_kernel_spmd

    if MODE not in _cache:
        _cache[MODE] = _build_nc(MODE)
    nc = _cache[MODE]
    in_maps = _prepare_in_maps(x, W, b)
    res = run_bass_kernel_spmd(nc, in_maps, list(range(NCORES)), trace=trace)
    out = np.concatenate([res.results[c]["spk"] for c in range(NCORES)], axis=2)
    return out, res


def kernel(x, W, b):
    out, _ = run(x, W, b, trace=False)
    return out


# revision 26
# speedup vs baseline: 1.0043x; 1.0043x over previous
"""Trainium2 Bass kernel for nn_FFNet_17600775979626.

Spiking FFN layer: cur = einsum('tbi,oi->tbo', x, W) + b, followed by a
leaky-integrate-and-fire scan over T with subtractive reset (snntorch Leaky,
beta=0.95, threshold=1.0). Returns spk_rec [T, B, NO] (0.0/1.0 floats).

Distribution: output-neuron sharding. Each of the 8 cores computes all
(T, B) for a 256-wide slice of the 2048 output neurons. x is replicated
(transposed on host so the contraction dim lands on SBUF partitions); W^T
and b are sliced per core. The per-timestep GEMM accumulates 16 K-chunk
matmuls (+ a K=1 bias seed matmul) into a PSUM tile laid out [b=128, o=256],
which is exactly the layout the membrane scan needs, so the scan reads PSUM
directly with full-width DVE ops and spikes DMA out contiguously.

Recurrence (TH=1): m_t = w_{t-1} + cur_t;  spk_t = (m_t > 1);
                   w_t = beta*m_t - spk_t.

Walrus codegen on this target accepts at most ONE sync-wait command per
engine instruction, while Tile's wait assigner freely emits several. Two
post-scheduling passes fix that: _slim_waits drops waits already implied
transitively (per-queue FIFO dispatch + semaphore vector clocks), and
_split_waits moves any excess waits onto injected same-queue NoOps.
"""

import os

import numpy as np

T, B, NI, NO = 128, 128, 2048, 2048
NCORES = 8
O_S = NO // NCORES  # 256 output neurons per core
KC = NI // 128  # 16 contraction chunks
BETA = 0.95

# GEMM modes:
#  fp16x2: x and W split into fp16 (hi, lo) pairs on host; three 1-cycle/row
#          passes xh*Wh + xh*Wl + xl*Wh. fp16 products are exact in fp32, so
#          accuracy matches native fp32 while streaming 3x faster.
#  fp32:   native fp32 matmuls (4 cycles/row).
#  f32r:   relaxed fp32, 1 cycle/row but ~tf32 multiply precision (~3e-2 rel
#          error on this problem's spikes) - too lossy, kept for experiments.
MODE = os.environ.get("KERNEL_MODE", "fp16x2")

_cache = {}


def _build_nc(mode):
    from contextlib import ExitStack

    import concourse.bass as bass
    import concourse.mybir as mybir
    import concourse.tile as tile

    f32 = mybir.dt.float32
    split16 = mode == "fp16x2"
    if split16:
        dt_mm = mybir.dt.float16
    elif mode == "f32r":
        dt_mm = mybir.dt.float32r
    else:
        dt_mm = f32

    nc = bass.Bass()
    # xT planes: fp16x2 ships (hi, lo); other modes use plane 0 only
    n_planes = 2 if split16 else 1
    xT = nc.declare_dram_parameter("xT", [n_planes, NI, T * B], dt_mm, isOutput=False)
    WTs = nc.declare_dram_parameter("WTs", [n_planes, NI, O_S], dt_mm, isOutput=False)
    # cols 0..127: ones (lhsT of K=1 bias seed matmuls); then bias plane(s)
    ob = nc.declare_dram_parameter(
        "ob", [1, 128 + n_planes * O_S], dt_mm, isOutput=False
    )
    spk = nc.declare_dram_parameter("spk", [T, B, O_S], f32, isOutput=True)

    TQ = 4  # timesteps per DMA batch (>=512B contiguous runs, fewer DMAs)
    assert T % TQ == 0

    with tile.TileContext(nc) as tc, ExitStack() as ctx:
        singles = ctx.enter_context(tc.tile_pool(name="singles", bufs=1))
        xpool = ctx.enter_context(tc.tile_pool(name="xp", bufs=2))
        spool = ctx.enter_context(tc.tile_pool(name="sp", bufs=3))
        psum = ctx.enter_context(tc.tile_pool(name="ps", bufs=6, space="PSUM"))

        xTr = xT[:].rearrange("h (k p) tb -> p h k tb", p=128)

        # Prefetch the first timestep batch before the W preload so the
        # pass-1 matmuls start as early as possible.
        xt0 = xpool.tile([128, n_planes, KC, TQ * B], dt_mm)
        nc.sync.dma_start(out=xt0[:], in_=xTr[:, :, :, : TQ * B])

        # W^T resident in SBUF: [i%128, plane, i//128, o]. Load per-plane
        # (hi first) so pass-1 matmuls can start before the lo plane lands.
        wt_sb = singles.tile([128, n_planes, KC, O_S], dt_mm)
        WTr = WTs[:].rearrange("h (k p) o -> p h k o", p=128)
        for h in range(n_planes):
            nc.sync.dma_start(out=wt_sb[:, h], in_=WTr[:, h])

        ob_sb = singles.tile([1, 128 + n_planes * O_S], dt_mm)
        nc.sync.dma_start(out=ob_sb[:], in_=ob[:])

        m_sb = singles.tile([128, O_S], f32)  # membrane potential
        w_sb = singles.tile([128, O_S], f32)  # carry: beta*m - spk
        nc.vector.memset(w_sb[:], 0.0)

        spk_r = spk[:].rearrange("(tq tt) b o -> tq b tt o", tt=TQ)

        for tq in range(T // TQ):
            if tq == 0:
                xt = xt0
            else:
                xt = xpool.tile([128, n_planes, KC, TQ * B], dt_mm)
                nc.sync.dma_start(
                    out=xt[:], in_=xTr[:, :, :, tq * TQ * B : (tq + 1) * TQ * B]
                )
            st = spool.tile([128, TQ, O_S], f32)

            for tt in range(TQ):
                ps = psum.tile([128, O_S], f32)
                first = True
                for k in range(KC):
                    if split16:
                        # xh*Wh, xh*Wl (shared stationary xh), then xl*Wh
                        passes = ((0, 0), (0, 1), (1, 0))
                    else:
                        passes = ((0, 0),)
                    for hx, hw in passes:
                        nc.tensor.matmul(
                            ps[:],
                            lhsT=xt[:, hx, k, tt * B : (tt + 1) * B],
                            rhs=wt_sb[:, hw, k, :],
                            start=first,
                            stop=False,
                        )
                        first = False
                for h in range(n_planes):
                    nc.tensor.matmul(
                        ps[:],
                        lhsT=ob_sb[:, :128],
                        rhs=ob_sb[:, 128 + h * O_S : 128 + (h + 1) * O_S],
                        start=False,
                        stop=(h == n_planes - 1),
                    )

                nc.vector.tensor_tensor(m_sb[:], w_sb[:], ps[:], mybir.AluOpType.add)
                nc.vector.tensor_scalar(
                    st[:, tt, :], m_sb[:], 1.0, None, mybir.AluOpType.is_gt
                )
                nc.vector.scalar_tensor_tensor(
                    w_sb[:],
                    m_sb[:],
                    BETA,
                    st[:, tt, :],
                    mybir.AluOpType.mult,
                    mybir.AluOpType.subtract,
                )
            # one store per TQ steps: dst [b part, tt, o] view of spk[tq]
            nc.sync.dma_start(out=spk_r[tq], in_=st[:])

    _slim_waits(nc)
    _split_waits(nc)
    return nc


def _slim_waits(nc):
    """Drop sync waits already implied by earlier ones (transitive closure).

    Each engine queue dispatches in FIFO order, so a wait satisfied on an
    earlier instruction of the same queue covers later instructions. A wait
    on sem s >= v also imports everything the incrementing instruction's
    queue had itself waited for when it raised s to v (semaphore vector
    clocks with snapshots at each increment).
    """
    FRAMEWORK_OPS = ("InstEventSemaphore", "InstDrain")
    engine_clock = {}  # engine -> {sem_id: value known reached}
    totals = {}  # sem_id -> running total of increments
    snapshots = {}  # sem_id -> [(value, clock dict)] in increasing value order
    poisoned = set()  # sems touched by non-monotonic updates (barriers)

    def join(dst, src):
        for s, v in src.items():
            if s in poisoned:
                continue
            if dst.get(s, -1) < v:
                dst[s] = v

    for blk in nc.m.functions[0].blocks:
        for inst in blk.instructions:
            si = getattr(inst, "sync_info", None)
            if si is None:
                continue
            is_framework = type(inst).__name__ in FRAMEWORK_OPS
            clock = engine_clock.setdefault(inst.engine, {})
            if si.on_wait:
                kept = []
                for w in si.on_wait:
                    if (
                        w.sync_type != "semaphore"
                        or w.wait_mode != "sem-ge-imm"
                        or w.id in poisoned
                    ):
                        kept.append(w)
                        continue
                    covered = clock.get(w.id, -1) >= w.wait_value
                    for val, snap in snapshots.get(w.id, ()):
                        if val <= w.wait_value:
                            join(clock, snap)
                        else:
                            break
                    if clock.get(w.id, -1) < w.wait_value:
                        clock[w.id] = w.wait_value
                    if is_framework or not covered:
                        kept.append(w)
                si.on_wait = kept
            if si.on_update:
                for u in si.on_update:
                    if u.sync_type != "semaphore":
                        continue
                    if u.update_mode not in ("sem-inc", "sem-add-imm"):
                        # barrier-style sem: stop reasoning about it entirely
                        poisoned.add(u.id)
                        totals.pop(u.id, None)
                        snapshots.pop(u.id, None)
                        for c in engine_clock.values():
                            c.pop(u.id, None)
                        continue
                    if u.id in poisoned:
                        continue
                    tot = totals.get(u.id, 0) + (u.update_value or 1)
                    totals[u.id] = tot
                    snap = dict(clock)
                    snap[u.id] = tot
                    snapshots.setdefault(u.id, []).append((tot, snap))


def _split_waits(nc, limit=1):
    """Move excess sync waits onto injected same-queue NoOps.

    Walrus codegen accepts at most `limit` sync-wait commands per engine
    instruction on this target. Engine queues dispatch in order, so a
    preceding NoOp carrying the wait is equivalent.
    """
    import concourse.mybir as mybir

    n_nops = 0
    for blk in nc.m.functions[0].blocks:
        out = []
        changed = False
        for inst in blk.instructions:
            si = getattr(inst, "sync_info", None)
            if type(inst).__name__ == "InstEventSemaphore":
                out.append(inst)
                continue
            if si is not None and si.on_wait and len(si.on_wait) > limit:
                waits = list(si.on_wait)
                for w in waits[:-limit]:
                    nop = mybir.InstNoOp(name=f"wnop-{n_nops}", ins=[], outs=[])
                    n_nops += 1
                    nop.engine = inst.engine
                    nop.sync_info = mybir.SyncInfo(on_wait=[w], on_update=[])
                    nop.bass_nofuse = True
                    out.append(nop)
                    changed = True
                si.on_wait = waits[-limit:]
            out.append(inst)
        if changed:
            try:
                blk.instructions = out
            except Exception:
                blk.instructions.clear()
                blk.instructions.extend(out)


def _split16(a):
    hi = a.astype(np.float16)
    lo = (a - hi.astype(np.float32)).astype(np.float16)
    return hi, lo


def _prepare_in_maps(x, W, b):
    x = np.ascontiguousarray(x, dtype=np.float32)
    W = np.ascontiguousarray(W, dtype=np.float32)
    b = np.ascontiguousarray(b, dtype=np.float32)
    # row tb = t*B + b so a 128-column block of xT = one full timestep
    x2 = x.reshape(T * B, NI)
    if MODE == "fp16x2":
        xh, xl = _split16(x2)
        xT = np.stack([np.ascontiguousarray(xh.T), np.ascontiguousarray(xl.T)])
        Wh, Wl = _split16(W)
        WTs_full = np.stack([np.ascontiguousarray(Wh.T), np.ascontiguousarray(Wl.T)])
        bh, bl = _split16(b)
        b_planes = [bh, bl]
        npdt = np.float16
    else:
        xT = np.ascontiguousarray(x2.T)[None]
        WTs_full = np.ascontiguousarray(W.T)[None]
        b_planes = [b]
        npdt = np.float32
    n_planes = len(b_planes)
    in_maps = []
    for c in range(NCORES):
        ob = np.empty((1, 128 + n_planes * O_S), npdt)
        ob[0, :128] = 1.0
        for h in range(n_planes):
            ob[0, 128 + h * O_S : 128 + (h + 1) * O_S] = b_planes[h][
                c * O_S : (c + 1) * O_S
            ]
        in_maps.append(
            {
                "xT": xT,
                "WTs": np.ascontiguousarray(WTs_full[:, :, c * O_S : (c + 1) * O_S]),
                "ob": ob,
            }
        )
    return in_maps


def run(x, W, b, trace=False):
    """Run the kernel; returns (out [T,B,NO] fp32, BassKernelResults)."""
    from concourse.bass_utils import run_bass_kernel_spmd

    if MODE not in _cache:
        _cache[MODE] = _build_nc(MODE)
    nc = _cache[MODE]
    in_maps = _prepare_in_maps(x, W, b)
    res = run_bass_kernel_spmd(nc, in_maps, list(range(NCORES)), trace=trace)
    out = np.concatenate([res.results[c]["spk"] for c in range(NCORES)], axis=2)
    return out, res


def kernel(x, W, b):
    out, _ = run(x, W, b, trace=False)
    return out
